# revision 46
# baseline (speedup 1.0000x reference)
"""Trainium2 Bass kernel for nn_NoPropDTEncoder (8-core data-parallel over batch).

v2 — SBUF-residency redesign (vs v1's DRAM-scratch streaming):
  - x is host-cast to bf16 and staged in natural layout; N_RES rows stay
    SBUF-resident for the whole kernel, the rest re-stream per pass
    (pool + 4 blocks).  No f32 x read, no on-device scratch writes.
  - x^T tiles (needed by the score matmuls, which contract over d) are
    rebuilt on the fly by PE transposes in per-dc sub-tiles; the PSUM
    evac is a single DVE copy per (row, dc).
  - The 20 big per-block matrices (wq/wk/wv/wo/cden x 4) are fp8
    (power-of-2 per-matrix scale, folded into the PSUM-evac activation
    scale) and streamed per block, double-buffered.  fp8 lhsT x bf16
    rhs mixed matmuls.
  - Score AND weighted-sum matmuls are column-tiled 4x (tile_position)
    so 4 rows' streams run concurrently on the PE.
  - z-path GEMMs stay batched over all 32 rows.

Layouts:
  - state zT etc: [128 part = d within block, 32 b, 6 dblk], f32
  - xn (natural) tiles [128 s, 4 sc, 768 d] bf16
  - xt sub-tiles [128 d, 512 s] bf16 per (row, dc), transient
  - cT compact [128 d, 6 dc, 8 g, 32 (bb,h)] bf16
"""
import sys
import os
import math

for _p in ("/opt/trn_rl_repo",):
    if _p not in sys.path and os.path.isdir(_p):
        sys.path.insert(0, _p)

import numpy as np
import concourse.bass as bass
import concourse.mybir as mybir
from concourse import tile
from concourse.bass_utils import run_bass_kernel_spmd

F32 = mybir.dt.float32
BF16 = mybir.dt.bfloat16
FP8 = mybir.dt.float8e4
WDT = FP8                 # dtype of the 20 big per-block matrices

B, S, D, H, DH, T, NCLS = 256, 512, 768, 8, 96, 4, 14
NCORES = 8
BL = B // NCORES          # 32 batch rows per core
DBLK = D // 128           # 6
SCH = S // 128            # 4
EPS = 1e-5
RSQD = 1.0 / math.sqrt(DH)

GB = 4                    # rows per column-tiled group
NGRP = BL // GB           # 8
N_RES = 8                 # SBUF-resident rows (rest streamed per pass)
N_SLOT = 8                # stream-buffer ring slots (2 groups deep for the
                          # phase1/phase2 software pipeline)
XT_RES = 4                # rows whose transposed layout is also resident
                          # (host-staged; skips their PE transposes)
GP = 32 * (GB - 1) + H    # 104 live partitions in a score/wsum group

AF = mybir.ActivationFunctionType

VKEYS = ("bo_sa", "projb_sd", "ln1_g", "ln1_b", "ln2_g", "ln2_b",
         "bn_s", "bn_b")


def split_sync_waits(nc, cap=1):
    """walrus in this container rejects >1 sync wait per CTRL instruction;
    move extra waits onto NoOp carriers inserted just before (same engine,
    program order => semantically identical)."""
    for f in nc.m.functions:
        for bb in f.blocks:
            il = bb.instructions
            i = 0
            while i < len(il):
                inst = il[i]
                si = inst.sync_info
                nw = len(si.on_wait) if si is not None else 0
                if nw > cap:
                    waits = list(si.on_wait)
                    ups = list(si.on_update)
                    extra, keep = waits[:-cap], waits[-cap:]
                    for j in range(0, len(extra), cap):
                        nop = mybir.InstNoOp(
                            name=f"{inst.name}-sw{j}", engine=inst.engine,
                            sync_info=mybir.SyncInfo(on_wait=extra[j:j + cap],
                                                     on_update=[]),
                            ins=[], outs=[])
                        il.insert(i, nop)
                        i += 1
                    inst.sync_info = mybir.SyncInfo(on_wait=keep, on_update=ups)
                i += 1


# ---------------------------------------------------------------------------
# host-side weight preprocessing
# ---------------------------------------------------------------------------

def _np_dt(dt):
    return mybir.dt.np(dt)


def _fp8_quant(a):
    """Quantize [*, n, m] f32 to WDT with a power-of-2 scale; returns
    (quantized array, inv_scale float)."""
    if WDT == BF16:
        return a.astype(_np_dt(BF16)), 1.0
    amax = float(np.abs(a).max())
    # ml_dtypes.float8_e4m3 max finite is 448; stay well below
    s = 2.0 ** math.floor(math.log2(192.0 / max(amax, 1e-30)))
    q = (a * s).astype(_np_dt(FP8))
    return q, 1.0 / s


def _prep_weights(inp):
    f = np.float32
    w = {}
    wq = np.asarray(inp["w_q"], f)
    wk = np.asarray(inp["w_k"], f)
    wv = np.asarray(inp["w_v"], f)
    wo = np.asarray(inp["w_o"], f)
    bqkv = np.asarray(inp["b_qkv"], f)
    proj_w = np.asarray(inp["proj_w"], f)
    proj_b = np.asarray(inp["proj_b"], f)
    sa = np.asarray(inp["scale_attn"], f)
    sd = np.asarray(inp["scale_denoise"], f)

    wqT = np.ascontiguousarray(np.transpose(wq, (0, 2, 1)) * RSQD)  # [T, j, i]
    bq = bqkv[:, :D] * RSQD
    w["bq"] = np.ascontiguousarray(
        bq.reshape(T, H, DH).transpose(0, 2, 1)).astype(f)          # [T, 96, 8]
    wvT = np.ascontiguousarray(np.transpose(wv, (0, 2, 1)))
    woT = np.ascontiguousarray(np.transpose(wo, (0, 2, 1)))         # [T, m, k]
    bv = bqkv[:, 2 * D:]
    bo_eff = np.asarray(inp["b_o"], f) + np.einsum("tkm,tm->tk", wo, bv)
    bo_sa = (sa[:, None] * bo_eff).astype(f)                        # [T, 768]

    # denoise: z_spatial = z1 @ M_t ; fold with proj -> C = M @ proj_w.T
    sr = np.asarray(inp["scale_real"], np.float64)
    si = np.asarray(inp["scale_imag"], np.float64)
    filt = np.mean(sr + 1j * si, axis=2)                            # [T, 16]
    jj = np.arange(D)
    mm = np.arange(16)
    W1 = np.exp(-2j * np.pi * np.outer(jj, mm) / D)                 # [768, 16]
    W2 = np.exp(+2j * np.pi * np.outer(mm, jj) / D)                 # [16, 768]
    cden = np.empty((T, D, D), np.float32)
    for t in range(T):
        Mt = np.real(W1 @ (filt[t][:, None] * W2)) / D              # [j, j']
        cden[t] = (Mt @ proj_w[t].T.astype(np.float64)).astype(np.float32)
    cden = cden * sd[:, None, None]                                 # [T, j, k]
    projb_sd = (sd[:, None] * proj_b).astype(f)

    # fp8 quantization per (matrix, block)
    inv = np.zeros((T, 5), f)
    q8 = {}
    for name, arr, idx in (("wqT", wqT, 0), ("wk", wk, 1), ("wvT", wvT, 2),
                           ("woT", woT, 3), ("cden", cden, 4)):
        qs = []
        for t in range(T):
            q, ivs = _fp8_quant(arr[t])
            qs.append(q)
            inv[t, idx] = ivs
        q8[name] = np.ascontiguousarray(np.stack(qs))
    w.update(q8)
    w["wscl"] = inv

    s = 1.0 / math.sqrt(1.0 + EPS)
    vec = {
        "bo_sa": bo_sa, "projb_sd": projb_sd,
        "ln1_g": np.asarray(inp["ln1_g"], f), "ln1_b": np.asarray(inp["ln1_b"], f),
        "ln2_g": np.asarray(inp["ln2_g"], f), "ln2_b": np.asarray(inp["ln2_b"], f),
        "bn_s": (np.asarray(inp["bn_g"], f) * s).astype(f),
        "bn_b": np.asarray(inp["bn_b"], f),
    }
    w["vecs8"] = np.ascontiguousarray(
        np.stack([vec[k] for k in VKEYS], axis=1))                  # [T, 8, 768]

    f1s = (np.asarray(inp["f1_bn_g"], f) * s).astype(f)
    w["f1w"] = np.asarray(inp["f1_w"], f)
    w["f1s"] = f1s
    w["f1b"] = (np.asarray(inp["f1_b"], f) * f1s + np.asarray(inp["f1_bn_b"], f))
    f2s = (np.asarray(inp["f2_bn_g"], f) * s).astype(f)
    w["f2w"] = np.asarray(inp["f2_w"], f)
    w["f2s"] = f2s
    w["f2b"] = (np.asarray(inp["f2_b"], f) * f2s + np.asarray(inp["f2_bn_b"], f))
    w["clsw"] = np.asarray(inp["cls_w"], f)
    w["clsb"] = np.asarray(inp["cls_b"], f)

    w["poolw"] = np.asarray(inp["pool_w"], f).reshape(D)
    w["ident"] = np.eye(128, dtype=np.float32)

    g = 1.0 / (1.0 + np.exp(-np.asarray(inp["gate"], np.float64)))
    scal = {"g": [float(v) for v in g],
            "sa": [float(v) for v in sa],
            "wscl": [[float(x) for x in row] for row in inv]}
    return w, scal


# ---------------------------------------------------------------------------
# program builder
# ---------------------------------------------------------------------------

def build_program(scal):
    nc = bass.Bass()
    P = {}
    P["xn"] = nc.declare_dram_parameter("xn", [BL, SCH, 128, D], BF16,
                                        isOutput=False)
    P["xt"] = nc.declare_dram_parameter("xt", [XT_RES, DBLK, 128, S], BF16,
                                        isOutput=False)
    P["ident"] = nc.declare_dram_parameter("ident", [128, 128], F32, isOutput=False)
    P["poolw"] = nc.declare_dram_parameter("poolw", [D], BF16, isOutput=False)
    P["wqT"] = nc.declare_dram_parameter("wqT", [T, D, D], WDT, isOutput=False)
    P["bq"] = nc.declare_dram_parameter("bq", [T, DH, H], F32, isOutput=False)
    P["wk"] = nc.declare_dram_parameter("wk", [T, D, D], WDT, isOutput=False)
    P["wvT"] = nc.declare_dram_parameter("wvT", [T, D, D], WDT, isOutput=False)
    P["woT"] = nc.declare_dram_parameter("woT", [T, D, D], WDT, isOutput=False)
    P["cden"] = nc.declare_dram_parameter("cden", [T, D, D], WDT, isOutput=False)
    P["vecs8"] = nc.declare_dram_parameter("vecs8", [T, 8, D], F32, isOutput=False)
    P["f1w"] = nc.declare_dram_parameter("f1w", [D, 256], BF16, isOutput=False)
    P["f1s"] = nc.declare_dram_parameter("f1s", [256], F32, isOutput=False)
    P["f1b"] = nc.declare_dram_parameter("f1b", [256], F32, isOutput=False)
    P["f2w"] = nc.declare_dram_parameter("f2w", [256, 128], BF16, isOutput=False)
    P["f2s"] = nc.declare_dram_parameter("f2s", [128], F32, isOutput=False)
    P["f2b"] = nc.declare_dram_parameter("f2b", [128], F32, isOutput=False)
    P["clsw"] = nc.declare_dram_parameter("clsw", [128, NCLS], BF16, isOutput=False)
    P["clsb"] = nc.declare_dram_parameter("clsb", [NCLS], F32, isOutput=False)
    P["out"] = nc.declare_dram_parameter("out", [NCLS, BL], F32, isOutput=True)

    with tile.TileContext(nc) as tc:
        _body(nc, tc, P, scal)
    split_sync_waits(nc)
    return nc


def _body(nc, tc, P, scal):
    import contextlib
    ctx = contextlib.ExitStack()
    pool_c = ctx.enter_context(tc.tile_pool(name="consts", bufs=1))
    pool_res = ctx.enter_context(tc.tile_pool(name="xres", bufs=1))
    pool_str = ctx.enter_context(tc.tile_pool(name="xstr", bufs=1))
    pool_xt = ctx.enter_context(tc.tile_pool(name="xt", bufs=2))
    pool_wA = ctx.enter_context(tc.tile_pool(name="wA", bufs=2))
    pool_wB = ctx.enter_context(tc.tile_pool(name="wB", bufs=2))
    pool_wC = ctx.enter_context(tc.tile_pool(name="wC", bufs=2))
    pool_s = ctx.enter_context(tc.tile_pool(name="state", bufs=1))
    pool_t = ctx.enter_context(tc.tile_pool(name="tmp", bufs=2))
    psum = ctx.enter_context(tc.tile_pool(name="ps", bufs=1,
                                          space=bass.MemorySpace.PSUM))

    def dma(dst, src):
        nc.sync.dma_start(out=dst, in_=src)

    # ---- constants -------------------------------------------------------
    ident = pool_c.tile([128, 128], F32, tag="ident")
    dma(ident[:], P["ident"][:])
    identb = pool_c.tile([128, 128], BF16, tag="identb")
    nc.vector.tensor_copy(identb[:], ident[:])
    poolw = pool_c.tile([128, DBLK], BF16, tag="poolw")
    dma(poolw[:], P["poolw"].rearrange("(c p) -> p c", p=128))
    onescol = pool_c.tile([128, 1], F32, tag="ones")
    nc.vector.memset(onescol[:], 1.0 / D)
    onesrow = pool_c.tile([1, 128], F32, tag="onesrow")
    nc.vector.memset(onesrow[:], 1.0)
    zeroc = pool_c.tile([128, 1], F32, tag="zeroc")
    nc.vector.memset(zeroc[:], 0.0)
    nc.const_aps.aps[(F32, 0.0)] = zeroc[:]
    epsc = pool_c.tile([128, 1], F32, tag="epsc")
    nc.vector.memset(epsc[:], EPS)
    nc.const_aps.aps[(F32, EPS)] = epsc[:]

    # classifier weights (loaded once)
    f1w = pool_c.tile([128, DBLK, 256], BF16, tag="f1w")
    dma(f1w[:], P["f1w"].rearrange("(c p) n -> p c n", p=128))
    f2w = pool_c.tile([128, 2, 128], BF16, tag="f2w")
    dma(f2w[:], P["f2w"].rearrange("(c p) n -> p c n", p=128))
    clsw = pool_c.tile([128, NCLS], BF16, tag="clsw")
    dma(clsw[:], P["clsw"][:])
    f1s = pool_c.tile([128, 2], F32, tag="f1s")
    dma(f1s[:], P["f1s"].rearrange("(c p) -> p c", p=128))
    f1b = pool_c.tile([128, 2], F32, tag="f1b")
    dma(f1b[:], P["f1b"].rearrange("(c p) -> p c", p=128))
    f2s = pool_c.tile([128, 1], F32, tag="f2s")
    dma(f2s[:], P["f2s"].rearrange("(c p) -> p c", p=128))
    f2b = pool_c.tile([128, 1], F32, tag="f2b")
    dma(f2b[:], P["f2b"].rearrange("(c p) -> p c", p=128))
    clsb = pool_c.tile([NCLS, 1], F32, tag="clsb")
    dma(clsb[:], P["clsb"].rearrange("(p c) -> p c", c=1))
    vecs = pool_c.tile([128, T, 8, DBLK], F32, tag="vecs")
    dma(vecs[:], P["vecs8"].rearrange("t v (c p) -> p t v c", p=128))
    bqv = pool_c.tile([DH, T, H], F32, tag="bqv")
    dma(bqv[:], P["bq"].rearrange("t p h -> p t h"))

    # persistent state
    zT = pool_s.tile([128, BL, DBLK], F32, tag="zT")
    zTb = pool_s.tile([128, BL, DBLK], BF16, tag="zTb")
    y1T = pool_s.tile([128, BL, DBLK], F32, tag="y1T")   # yT then z1T home
    y2T = pool_s.tile([128, BL, DBLK], F32, tag="y2T")   # y2T then z2T home
    z1Tb = pool_s.tile([128, BL, DBLK], BF16, tag="z1Tb")
    logitsT = pool_s.tile([NCLS, BL], F32, tag="logits")
    cT = pool_s.tile([128, DBLK, NGRP, GB * H], BF16, tag="cT")

    # resident xn rows (load now, keep forever)
    resident = {}
    for b in range(N_RES):
        xn = pool_res.tile([128, SCH, D], BF16, tag=f"xnr{b}", name=f"xnr{b}")
        dma(xn[:], P["xn"][b].rearrange("sc p d -> p sc d"))
        resident[b] = xn
    # rows with the transposed layout also resident (no per-pass transposes)
    xtres = {}
    for b in range(XT_RES):
        xtr = pool_res.tile([128, DBLK, S], BF16, tag=f"xtr{b}", name=f"xtr{b}")
        dma(xtr[:], P["xt"][b].rearrange("dc p s -> p dc s"))
        xtres[b] = xtr

    def vslice(t, k, dc):
        return vecs[:, t, VKEYS.index(k), dc:dc + 1]

    # ------------------------------------------------------------------
    # helpers
    # ------------------------------------------------------------------

    def ensure_xn(b):
        if b in resident:
            return resident[b]
        xn = pool_str.tile([128, SCH, D], BF16, tag=f"xs{b % N_SLOT}",
                           name=f"xs{b}")
        dma(xn[:], P["xn"][b].rearrange("sc p d -> p sc d"))
        return xn

    def make_xt_slice(xns, gi, dc):
        """PE-transpose dc-slice of 4 rows -> 4 xt sub-tiles [128, 512]."""
        outs = []
        for bb in range(GB):
            pt = psum.tile([128, SCH, 128], BF16, tag=f"tr{bb % 2}",
                           bufs=1)
            for sc in range(SCH):
                nc.tensor.transpose(pt[:, sc, :],
                                    xns[bb][:, sc, dc * 128:(dc + 1) * 128],
                                    identb[:])
            xts = pool_xt.tile([128, 512], BF16, tag=f"xts{bb}",
                               name=f"xt_g{gi}_d{dc}_b{bb}", bufs=3)
            nc.vector.tensor_copy(xts[:], pt[:].rearrange("p sc c -> p (sc c)"))
            outs.append(xts)
        return outs

    def stream_phase1(gi, score_lhsT, pool_mode):
        """Transposes + col-tiled scores + softmax for rows gi*4..gi*4+3.
        Emitted a group AHEAD of phase2 so the PE stays busy during the
        scalar/DVE softmax chain of the previous group."""
        bs = list(range(gi * GB, (gi + 1) * GB))
        xns = [ensure_xn(b) for b in bs]
        M = 1 if pool_mode else H
        ps = psum.tile([128, S], F32, tag="sc", bufs=2)
        for dc in range(DBLK):
            if bs[0] in xtres:
                xts = [xtres[b][:, dc, :] for b in bs]
            else:
                xts = [x[:] for x in make_xt_slice(xns, gi, dc)]
            for bb in range(GB):
                nc.tensor.matmul(ps[32 * bb:32 * bb + M, :],
                                 score_lhsT(dc, bb, bs[bb]), xts[bb],
                                 start=(dc == 0), stop=(dc == DBLK - 1),
                                 tile_position=(0, 32 * bb))
        gp = 32 * (GB - 1) + M
        es = pool_t.tile([GP, S], BF16, tag="att_e")
        den = pool_t.tile([GP, 2], F32, tag="att_d")
        nc.scalar.activation(es[:gp, :], ps[0:gp, :], AF.Exp,
                             accum_out=den[:gp, 0:1])
        nc.vector.reciprocal(den[:gp, 1:2], den[:gp, 0:1])
        att = pool_t.tile([GP, S], BF16, tag="att_n")
        nc.vector.tensor_scalar_mul(att[:gp, :], es[:gp, :], den[:gp, 1:2])
        return dict(gi=gi, att=att, xns=xns, M=M, gp=gp)

    def stream_phase2(st):
        """attT transpose + col-tiled weighted sums + cT-transpose evac.
        Returns ctp psum tile [128, DBLK, GP] bf16 (cols 32*bb+m live)."""
        att, xns, M, gp = st["att"], st["xns"], st["M"], st["gp"]
        ptr = psum.tile([128, SCH, GP], BF16, tag="ctr", bufs=1)
        for sc in range(SCH):
            nc.tensor.transpose(ptr[:, sc, :gp],
                                att[:gp, sc * 128:(sc + 1) * 128],
                                identb[:gp, :gp])
        attT = pool_t.tile([128, SCH, GP], BF16, tag="attT")
        nc.vector.tensor_copy(attT[:, :, :gp], ptr[:, :, :gp])
        c1 = psum.tile([128, 512], F32, tag="ws1", bufs=1)
        c2 = psum.tile([128, 256], F32, tag="ws2", bufs=1)
        for sc in range(SCH):
            for bb in range(GB):
                nc.tensor.matmul(c1[32 * bb:32 * bb + M, :],
                                 attT[:, sc, 32 * bb:32 * bb + M],
                                 xns[bb][:, sc, 0:512],
                                 start=(sc == 0), stop=(sc == SCH - 1),
                                 tile_position=(0, 32 * bb))
                nc.tensor.matmul(c2[32 * bb:32 * bb + M, :],
                                 attT[:, sc, 32 * bb:32 * bb + M],
                                 xns[bb][:, sc, 512:D],
                                 start=(sc == 0), stop=(sc == SCH - 1),
                                 tile_position=(0, 32 * bb))
        return evac_group_T(st["gi"], c1, c2, gp)

    def evac_group_T(gi, c1, c2, gp):
        """Evac c1/c2 [gp,512/256] -> sbuf bf16, transpose per dc -> psum;
        returns ctp psum tile [128, DBLK, GP] bf16 (cols 32*bb+m live)."""
        ch = pool_t.tile([GP, D], BF16, tag="chat")
        nc.vector.tensor_copy(ch[:gp, 0:512], c1[:gp, :])
        ctp = psum.tile([128, DBLK, GP], BF16, tag="ctr", bufs=1)
        # transposes for d<512 only need the c1 half; c2 evacs in parallel
        for dc in range(4):
            nc.tensor.transpose(ctp[:, dc, :gp],
                                ch[:gp, dc * 128:(dc + 1) * 128],
                                identb[:gp, :gp])
        nc.vector.tensor_copy(ch[:gp, 512:D], c2[:gp, :])
        for dc in range(4, DBLK):
            nc.tensor.transpose(ctp[:, dc, :gp],
                                ch[:gp, dc * 128:(dc + 1) * 128],
                                identb[:gp, :gp])
        return ctp

    # ==================================================================
    # pool pass (groups software-pipelined: phase1(g+1) before phase2(g))
    # ==================================================================
    def finish_pool(st):
        ctp = stream_phase2(st)
        for bb in range(GB):
            nc.vector.tensor_copy(zT[:, st["gi"] * GB + bb, :],
                                  ctp[:, :, 32 * bb])

    prev = None
    for gi in range(NGRP):
        cur = stream_phase1(gi, lambda dc, bb, b: poolw[:, dc:dc + 1],
                            pool_mode=True)
        if prev is not None:
            finish_pool(prev)
        prev = cur
    finish_pool(prev)

    nc.vector.tensor_copy(zTb[:], zT[:])

    # ==================================================================
    # transformer blocks
    # ==================================================================
    wscl = scal["wscl"]
    for t in range(T):
        g = scal["g"][t]
        sa = scal["sa"][t]
        s_q, s_k, s_v, s_o, s_c = wscl[t]

        # --- stream per-block weights (double-buffered via pools) ---
        wq = pool_wA.tile([128, DBLK, D], WDT, tag="wjq", name=f"wq{t}")
        dma(wq[:], P["wqT"][t].rearrange("(c p) n -> p c n", p=128))
        wk = pool_wB.tile([DH, H, D], WDT, tag="khk", name=f"wk{t}")
        dma(wk[:], P["wk"][t].rearrange("(h p) n -> p h n", p=DH))

        # --- q^T = wqT^T @ zT + bq  -> [96, 8, 32] ---
        qp = psum.tile([DH, H, BL], F32, tag="mm")
        for h in range(H):
            for jc in range(DBLK):
                nc.tensor.matmul(qp[:, h, :],
                                 wq[:, jc, h * DH:(h + 1) * DH],
                                 zTb[:, :, jc],
                                 start=(jc == 0), stop=(jc == DBLK - 1))
        qT = pool_t.tile([DH, H, BL], BF16, tag="qT", bufs=1)
        for h in range(H):
            nc.scalar.activation(qT[:, h, :], qp[:, h, :], AF.Identity,
                                 bias=bqv[:, t, h:h + 1], scale=s_q)

        # --- u = wk_h^T q_h -> [128, 6 dc, 8 h, 32 b], cast bf16 ---
        uT = pool_t.tile([128, DBLK, H, BL], BF16, tag="uT", bufs=1)
        for dc in range(DBLK):
            up = psum.tile([128, H, BL], F32, tag="mm")
            for h in range(H):
                nc.tensor.matmul(up[:, h, :],
                                 wk[:, h, dc * 128:(dc + 1) * 128],
                                 qT[:, h, :], start=True, stop=True)
            nc.scalar.activation(uT[:, dc, :, :], up[:], AF.Identity,
                                 scale=s_k)

        # prefetch tail weights while streaming
        wv = pool_wA.tile([128, DBLK, D], WDT, tag="wjq", name=f"wv{t}")
        dma(wv[:], P["wvT"][t].rearrange("(c p) n -> p c n", p=128))
        wo = pool_wB.tile([DH, H, D], WDT, tag="khk", name=f"wo{t}")
        dma(wo[:], P["woT"][t].rearrange("(h p) n -> p h n", p=DH))
        cdn = pool_wC.tile([128, DBLK, D], WDT, tag="cd", name=f"cdn{t}")
        dma(cdn[:], P["cden"][t].rearrange("(c p) n -> p c n", p=128))

        # --- streaming pass over x (software-pipelined groups) ---
        def finish_blk(st):
            ctp = stream_phase2(st)
            for bb in range(GB):
                nc.vector.tensor_copy(
                    cT[:, :, st["gi"], bb * H:(bb + 1) * H],
                    ctp[:, :, 32 * bb:32 * bb + H])

        prev = None
        for gi in range(NGRP):
            cur = stream_phase1(gi, lambda dc, bb, b: uT[:, dc, :, b],
                                pool_mode=False)
            if prev is not None:
                finish_blk(prev)
            prev = cur
        finish_blk(prev)

        # --- o_h = wvT_h^T @ c_h -> [96, 8, 32] ---
        op = psum.tile([DH, H, BL], F32, tag="mm")
        for h in range(H):
            for jc in range(DBLK):
                nc.tensor.matmul(
                    op[:, h, :],
                    wv[:, jc, h * DH:(h + 1) * DH],
                    cT[:, jc, :, :].rearrange("p g (bb h) -> p (g bb) h", bb=GB)[:, :, h],
                    start=(jc == 0), stop=(jc == DBLK - 1))
        oT = pool_t.tile([DH, H, BL], BF16, tag="oT", bufs=1)
        nc.scalar.activation(oT[:], op[:], AF.Identity, scale=s_v)

        # --- z_attn^T = woT^T @ o  (+ sa*bo_eff, * sa), y = z + ... ---
        zap = psum.tile([128, DBLK, BL], F32, tag="mm")
        for mk in range(DBLK):
            for h in range(H):
                nc.tensor.matmul(zap[:, mk, :],
                                 wo[:, h, mk * 128:(mk + 1) * 128],
                                 oT[:, h, :], start=(h == 0), stop=(h == H - 1))
        for mk in range(DBLK):
            nc.scalar.activation(y1T[:, :, mk], zap[:, mk, :], AF.Identity,
                                 bias=vslice(t, "bo_sa", mk), scale=sa * s_o)
        nc.vector.tensor_add(y1T[:], y1T[:], zT[:])

        # --- LN1 -> z1T (in place over y1T) ---
        _layernorm(nc, tc, psum, pool_t, y1T, y1T, onescol, onesrow,
                   lambda dc: vslice(t, "ln1_g", dc),
                   lambda dc: vslice(t, "ln1_b", dc))
        z1T = y1T
        nc.vector.tensor_copy(z1Tb[:], z1T[:])

        # --- denoise: z_den^T = cden^T @ z1T ; y2 = z1 + sd*(...) ---
        dp = psum.tile([128, DBLK, BL], F32, tag="mm")
        for mk in range(DBLK):
            for jc in range(DBLK):
                nc.tensor.matmul(dp[:, mk, :],
                                 cdn[:, jc, mk * 128:(mk + 1) * 128],
                                 z1Tb[:, :, jc],
                                 start=(jc == 0), stop=(jc == DBLK - 1))
        for mk in range(DBLK):
            nc.scalar.activation(y2T[:, :, mk], dp[:, mk, :], AF.Identity,
                                 bias=vslice(t, "projb_sd", mk), scale=s_c)
        nc.vector.tensor_add(y2T[:], y2T[:], z1T[:])

        # --- LN2 -> z2T (in place over y2T) ---
        _layernorm(nc, tc, psum, pool_t, y2T, y2T, onescol, onesrow,
                   lambda dc: vslice(t, "ln2_g", dc),
                   lambda dc: vslice(t, "ln2_b", dc))
        z2T = y2T

        # --- gate mix + BN -> new z ---
        nc.vector.tensor_sub(z2T[:], z2T[:], zT[:])       # z2 - z
        nc.vector.tensor_scalar(z2T[:], z2T[:], g, None,
                                op0=mybir.AluOpType.mult)  # g*(z2-z)
        nc.vector.tensor_add(z2T[:], z2T[:], zT[:])       # + z
        for dc in range(DBLK):
            nc.scalar.activation(zT[:, :, dc], z2T[:, :, dc], AF.Identity,
                                 bias=vslice(t, "bn_b", dc),
                                 scale=vslice(t, "bn_s", dc))

        # --- classifier ---
        nc.vector.tensor_copy(zTb[:], zT[:])
        hp = psum.tile([128, 2, BL], F32, tag="mm")
        for mk in range(2):
            for jc in range(DBLK):
                nc.tensor.matmul(hp[:, mk, :],
                                 f1w[:, jc, mk * 128:(mk + 1) * 128],
                                 zTb[:, :, jc],
                                 start=(jc == 0), stop=(jc == DBLK - 1))
        h1 = pool_t.tile([128, 2, BL], BF16, tag="h1")
        for mk in range(2):
            nc.scalar.activation(h1[:, mk, :], hp[:, mk, :], AF.Relu,
                                 bias=f1b[:, mk:mk + 1], scale=f1s[:, mk:mk + 1])
        h2p = psum.tile([128, BL], F32, tag="mm")
        for jc in range(2):
            nc.tensor.matmul(h2p[:], f2w[:, jc, :], h1[:, jc, :],
                             start=(jc == 0), stop=(jc == 1))
        h2 = pool_t.tile([128, BL], BF16, tag="h2")
        nc.scalar.activation(h2[:], h2p[:], AF.Relu,
                             bias=f2b[:, 0:1], scale=f2s[:, 0:1])
        lp = psum.tile([NCLS, BL], F32, tag="mm")
        nc.tensor.matmul(lp[:], clsw[:], h2[:], start=True, stop=True)
        if t == 0:
            nc.vector.tensor_copy(logitsT[:], lp[:])
        else:
            nc.vector.tensor_add(logitsT[:], logitsT[:], lp[:])

    # --- epilogue: /T + cls_b, store ---
    outt = pool_t.tile([NCLS, BL], F32, tag="outt")
    nc.scalar.activation(outt[:], logitsT[:], AF.Identity,
                         bias=clsb[:, 0:1], scale=1.0 / T)
    dma(P["out"][:], outt[:])
    ctx.close()


def _layernorm(nc, tc, psum, pool_t, yT, outT, onescol, onesrow, gf, bf):
    """T-layout layernorm over d (partition x dblk)."""
    lnp = psum.tile([1, 2, BL], F32, tag="mm")
    mp = lnp[:, 0, :]
    m2p = lnp[:, 1, :]
    sq = pool_t.tile([128, BL, DBLK], F32, tag="ln_sq", bufs=1)
    nc.scalar.square(sq[:], yT[:])
    # NOTE: mp and m2p share a PSUM bank row; interleaving their
    # accumulation groups is illegal (start=True clears has_written for
    # the whole partition-row of the bank) -> run mp's group to
    # completion before starting m2p's.
    for dc in range(DBLK):
        nc.tensor.matmul(mp[:], onescol[:], yT[:, :, dc],
                         start=(dc == 0), stop=(dc == DBLK - 1))
    for dc in range(DBLK):
        nc.tensor.matmul(m2p[:], onescol[:], sq[:, :, dc],
                         start=(dc == 0), stop=(dc == DBLK - 1))
    st = pool_t.tile([1, 2 * BL], F32, tag="ln_st")  # [mu | rstd]
    nc.vector.tensor_copy(st[:, 0:BL], mp[:])
    mu2 = pool_t.tile([1, BL], F32, tag="ln_mu2")
    nc.scalar.square(mu2[:], st[:, 0:BL])
    var = pool_t.tile([1, BL], F32, tag="ln_var")
    nc.vector.tensor_sub(var[:], m2p[:], mu2[:])
    nc.scalar.activation(var[:], var[:], AF.Sqrt, bias=EPS)
    nc.vector.reciprocal(st[:, BL:2 * BL], var[:])
    bcp = psum.tile([128, 2 * BL], F32, tag="ctr", bufs=1)
    nc.tensor.matmul(bcp[:], onesrow[:], st[:], start=True, stop=True)
    bc = pool_t.tile([128, 2 * BL], F32, tag="ln_bc")
    nc.vector.tensor_copy(bc[:], bcp[:])
    mub = bc[:, 0:BL]
    rsb = bc[:, BL:2 * BL]
    for dc in range(DBLK):
        nc.vector.tensor_sub(outT[:, :, dc], yT[:, :, dc], mub)
        nc.vector.tensor_mul(outT[:, :, dc], outT[:, :, dc], rsb)
        nc.scalar.activation(outT[:, :, dc], outT[:, :, dc], AF.Identity,
                             bias=bf(dc), scale=gf(dc))


# ---------------------------------------------------------------------------
# entry point
# ---------------------------------------------------------------------------

_PROG_CACHE = {}
LAST_EXEC_NS = None
LAST_RESULT = None


def _make_in_maps(inputs, w, scal):
    x = np.asarray(inputs["x_feat"], np.float32)
    assert x.shape == (B, S, D), x.shape
    bf = _np_dt(BF16)

    shared = {
        "ident": w["ident"],
        "poolw": w["poolw"].astype(bf),
        "wqT": w["wqT"], "bq": w["bq"], "wk": w["wk"],
        "wvT": w["wvT"], "woT": w["woT"], "cden": w["cden"],
        "vecs8": w["vecs8"],
        "f1w": w["f1w"].astype(bf), "f1s": w["f1s"], "f1b": w["f1b"],
        "f2w": w["f2w"].astype(bf), "f2s": w["f2s"], "f2b": w["f2b"],
        "clsw": w["clsw"].astype(bf), "clsb": w["clsb"],
    }
    in_maps = []
    for c in range(NCORES):
        m = dict(shared)
        xc = x[c * BL:(c + 1) * BL].astype(bf)            # [BL, S, D]
        m["xn"] = np.ascontiguousarray(
            xc.reshape(BL, SCH, 128, D))
        m["xt"] = np.ascontiguousarray(
            xc[:XT_RES].transpose(0, 2, 1).reshape(XT_RES, DBLK, 128, S))
        in_maps.append(m)
    return in_maps


def kernel(**inputs):
    global LAST_EXEC_NS, LAST_RESULT
    w, scal = _prep_weights(inputs)
    key = (tuple(scal["g"]) + tuple(scal["sa"])
           + tuple(x for row in scal["wscl"] for x in row))
    if key not in _PROG_CACHE:
        _PROG_CACHE[key] = build_program(scal)
    nc = _PROG_CACHE[key]
    in_maps = _make_in_maps(inputs, w, scal)
    res = run_bass_kernel_spmd(nc, in_maps, core_ids=list(range(NCORES)))
    LAST_RESULT = res
    if getattr(res, "exec_time_ns", None):
        LAST_EXEC_NS = res.exec_time_ns
    out = np.concatenate(
        [np.asarray(res.results[c]["out"]).T for c in range(NCORES)], axis=0)
    return out.astype(np.float32)


# revision 47
# speedup vs baseline: 1.1739x; 1.1739x over previous
"""Trainium2 Bass kernel for nn_NoPropDTEncoder (8-core data-parallel over batch).

v2 — SBUF-residency redesign (vs v1's DRAM-scratch streaming):
  - x is host-cast to bf16 and staged in natural layout; N_RES rows stay
    SBUF-resident for the whole kernel, the rest re-stream per pass
    (pool + 4 blocks).  No f32 x read, no on-device scratch writes.
  - x^T tiles (needed by the score matmuls, which contract over d) are
    rebuilt on the fly by PE transposes in per-dc sub-tiles; the PSUM
    evac is a single DVE copy per (row, dc).
  - The 20 big per-block matrices (wq/wk/wv/wo/cden x 4) are fp8
    (power-of-2 per-matrix scale, folded into the PSUM-evac activation
    scale) and streamed per block, double-buffered.  fp8 lhsT x bf16
    rhs mixed matmuls.
  - Score AND weighted-sum matmuls are column-tiled 4x (tile_position)
    so 4 rows' streams run concurrently on the PE.
  - z-path GEMMs stay batched over all 32 rows.

Layouts:
  - state zT etc: [128 part = d within block, 32 b, 6 dblk], f32
  - xn (natural) tiles [128 s, 4 sc, 768 d] bf16
  - xt sub-tiles [128 d, 512 s] bf16 per (row, dc), transient
  - cT compact [128 d, 6 dc, 8 g, 32 (bb,h)] bf16
"""
import sys
import os
import math

for _p in ("/opt/trn_rl_repo",):
    if _p not in sys.path and os.path.isdir(_p):
        sys.path.insert(0, _p)

import numpy as np
import concourse.bass as bass
import concourse.mybir as mybir
from concourse import tile
from concourse.bass_utils import run_bass_kernel_spmd

F32 = mybir.dt.float32
BF16 = mybir.dt.bfloat16
FP8 = mybir.dt.float8e4
WDT = FP8                 # dtype of the 20 big per-block matrices

B, S, D, H, DH, T, NCLS = 256, 512, 768, 8, 96, 4, 14
NCORES = 8
BL = B // NCORES          # 32 batch rows per core
DBLK = D // 128           # 6
SCH = S // 128            # 4
EPS = 1e-5
RSQD = 1.0 / math.sqrt(DH)

GB = 4                    # rows per column-tiled group
NGRP = BL // GB           # 8
N_RES = 8                 # SBUF-resident rows (rest streamed per pass)
N_SLOT = 8                # stream-buffer ring slots (2 groups deep for the
                          # phase1/phase2 software pipeline)
XT_RES = 4                # rows whose transposed layout is also resident
                          # (host-staged; skips their PE transposes)
GP = 32 * (GB - 1) + H    # 104 live partitions in a score/wsum group

AF = mybir.ActivationFunctionType

VKEYS = ("bo_sa", "projb_sd", "ln1_g", "ln1_b", "ln2_g", "ln2_b",
         "bn_s", "bn_b")


def split_sync_waits(nc, cap=1):
    """walrus in this container rejects >1 sync wait per CTRL instruction;
    move extra waits onto NoOp carriers inserted just before (same engine,
    program order => semantically identical)."""
    for f in nc.m.functions:
        for bb in f.blocks:
            il = bb.instructions
            i = 0
            while i < len(il):
                inst = il[i]
                si = inst.sync_info
                nw = len(si.on_wait) if si is not None else 0
                if nw > cap:
                    waits = list(si.on_wait)
                    ups = list(si.on_update)
                    extra, keep = waits[:-cap], waits[-cap:]
                    for j in range(0, len(extra), cap):
                        nop = mybir.InstNoOp(
                            name=f"{inst.name}-sw{j}", engine=inst.engine,
                            sync_info=mybir.SyncInfo(on_wait=extra[j:j + cap],
                                                     on_update=[]),
                            ins=[], outs=[])
                        il.insert(i, nop)
                        i += 1
                    inst.sync_info = mybir.SyncInfo(on_wait=keep, on_update=ups)
                i += 1


# ---------------------------------------------------------------------------
# host-side weight preprocessing
# ---------------------------------------------------------------------------

def _np_dt(dt):
    return mybir.dt.np(dt)


def _fp8_quant(a):
    """Quantize [*, n, m] f32 to WDT with a power-of-2 scale; returns
    (quantized array, inv_scale float)."""
    if WDT == BF16:
        return a.astype(_np_dt(BF16)), 1.0
    amax = float(np.abs(a).max())
    # ml_dtypes.float8_e4m3 max finite is 448; stay well below
    s = 2.0 ** math.floor(math.log2(192.0 / max(amax, 1e-30)))
    q = (a * s).astype(_np_dt(FP8))
    return q, 1.0 / s


def _prep_weights(inp):
    f = np.float32
    w = {}
    wq = np.asarray(inp["w_q"], f)
    wk = np.asarray(inp["w_k"], f)
    wv = np.asarray(inp["w_v"], f)
    wo = np.asarray(inp["w_o"], f)
    bqkv = np.asarray(inp["b_qkv"], f)
    proj_w = np.asarray(inp["proj_w"], f)
    proj_b = np.asarray(inp["proj_b"], f)
    sa = np.asarray(inp["scale_attn"], f)
    sd = np.asarray(inp["scale_denoise"], f)

    wqT = np.ascontiguousarray(np.transpose(wq, (0, 2, 1)) * RSQD)  # [T, j, i]
    bq = bqkv[:, :D] * RSQD
    w["bq"] = np.ascontiguousarray(
        bq.reshape(T, H, DH).transpose(0, 2, 1)).astype(f)          # [T, 96, 8]
    wvT = np.ascontiguousarray(np.transpose(wv, (0, 2, 1)))
    woT = np.ascontiguousarray(np.transpose(wo, (0, 2, 1)))         # [T, m, k]
    bv = bqkv[:, 2 * D:]
    bo_eff = np.asarray(inp["b_o"], f) + np.einsum("tkm,tm->tk", wo, bv)
    bo_sa = (sa[:, None] * bo_eff).astype(f)                        # [T, 768]

    # denoise: z_spatial = z1 @ M_t ; fold with proj -> C = M @ proj_w.T
    sr = np.asarray(inp["scale_real"], np.float64)
    si = np.asarray(inp["scale_imag"], np.float64)
    filt = np.mean(sr + 1j * si, axis=2)                            # [T, 16]
    jj = np.arange(D)
    mm = np.arange(16)
    W1 = np.exp(-2j * np.pi * np.outer(jj, mm) / D)                 # [768, 16]
    W2 = np.exp(+2j * np.pi * np.outer(mm, jj) / D)                 # [16, 768]
    cden = np.empty((T, D, D), np.float32)
    for t in range(T):
        Mt = np.real(W1 @ (filt[t][:, None] * W2)) / D              # [j, j']
        cden[t] = (Mt @ proj_w[t].T.astype(np.float64)).astype(np.float32)
    cden = cden * sd[:, None, None]                                 # [T, j, k]
    projb_sd = (sd[:, None] * proj_b).astype(f)

    # fp8 quantization per (matrix, block)
    inv = np.zeros((T, 5), f)
    q8 = {}
    for name, arr, idx in (("wqT", wqT, 0), ("wk", wk, 1), ("wvT", wvT, 2),
                           ("woT", woT, 3), ("cden", cden, 4)):
        qs = []
        for t in range(T):
            q, ivs = _fp8_quant(arr[t])
            qs.append(q)
            inv[t, idx] = ivs
        q8[name] = np.ascontiguousarray(np.stack(qs))
    w.update(q8)
    w["wscl"] = inv

    s = 1.0 / math.sqrt(1.0 + EPS)
    vec = {
        "bo_sa": bo_sa, "projb_sd": projb_sd,
        "ln1_g": np.asarray(inp["ln1_g"], f), "ln1_b": np.asarray(inp["ln1_b"], f),
        "ln2_g": np.asarray(inp["ln2_g"], f), "ln2_b": np.asarray(inp["ln2_b"], f),
        "bn_s": (np.asarray(inp["bn_g"], f) * s).astype(f),
        "bn_b": np.asarray(inp["bn_b"], f),
    }
    w["vecs8"] = np.ascontiguousarray(
        np.stack([vec[k] for k in VKEYS], axis=1))                  # [T, 8, 768]

    f1s = (np.asarray(inp["f1_bn_g"], f) * s).astype(f)
    w["f1w"] = np.asarray(inp["f1_w"], f)
    w["f1s"] = f1s
    w["f1b"] = (np.asarray(inp["f1_b"], f) * f1s + np.asarray(inp["f1_bn_b"], f))
    f2s = (np.asarray(inp["f2_bn_g"], f) * s).astype(f)
    w["f2w"] = np.asarray(inp["f2_w"], f)
    w["f2s"] = f2s
    w["f2b"] = (np.asarray(inp["f2_b"], f) * f2s + np.asarray(inp["f2_bn_b"], f))
    w["clsw"] = np.asarray(inp["cls_w"], f)
    w["clsb"] = np.asarray(inp["cls_b"], f)

    w["poolw"] = np.asarray(inp["pool_w"], f).reshape(D)
    w["ident"] = np.eye(128, dtype=np.float32)

    g = 1.0 / (1.0 + np.exp(-np.asarray(inp["gate"], np.float64)))
    scal = {"g": [float(v) for v in g],
            "sa": [float(v) for v in sa],
            "wscl": [[float(x) for x in row] for row in inv]}
    return w, scal


# ---------------------------------------------------------------------------
# program builder
# ---------------------------------------------------------------------------

def build_program(scal):
    nc = bass.Bass()
    P = {}
    P["xn"] = nc.declare_dram_parameter("xn", [BL, SCH, 128, D], BF16,
                                        isOutput=False)
    P["xt"] = nc.declare_dram_parameter("xt", [XT_RES, DBLK, 128, S], BF16,
                                        isOutput=False)
    P["ident"] = nc.declare_dram_parameter("ident", [128, 128], F32, isOutput=False)
    P["poolw"] = nc.declare_dram_parameter("poolw", [D], BF16, isOutput=False)
    P["wqT"] = nc.declare_dram_parameter("wqT", [T, D, D], WDT, isOutput=False)
    P["bq"] = nc.declare_dram_parameter("bq", [T, DH, H], F32, isOutput=False)
    P["wk"] = nc.declare_dram_parameter("wk", [T, D, D], WDT, isOutput=False)
    P["wvT"] = nc.declare_dram_parameter("wvT", [T, D, D], WDT, isOutput=False)
    P["woT"] = nc.declare_dram_parameter("woT", [T, D, D], WDT, isOutput=False)
    P["cden"] = nc.declare_dram_parameter("cden", [T, D, D], WDT, isOutput=False)
    P["vecs8"] = nc.declare_dram_parameter("vecs8", [T, 8, D], F32, isOutput=False)
    P["f1w"] = nc.declare_dram_parameter("f1w", [D, 256], BF16, isOutput=False)
    P["f1s"] = nc.declare_dram_parameter("f1s", [256], F32, isOutput=False)
    P["f1b"] = nc.declare_dram_parameter("f1b", [256], F32, isOutput=False)
    P["f2w"] = nc.declare_dram_parameter("f2w", [256, 128], BF16, isOutput=False)
    P["f2s"] = nc.declare_dram_parameter("f2s", [128], F32, isOutput=False)
    P["f2b"] = nc.declare_dram_parameter("f2b", [128], F32, isOutput=False)
    P["clsw"] = nc.declare_dram_parameter("clsw", [128, NCLS], BF16, isOutput=False)
    P["clsb"] = nc.declare_dram_parameter("clsb", [NCLS], F32, isOutput=False)
    P["out"] = nc.declare_dram_parameter("out", [NCLS, BL], F32, isOutput=True)

    with tile.TileContext(nc) as tc:
        _body(nc, tc, P, scal)
    split_sync_waits(nc)
    return nc


def _body(nc, tc, P, scal):
    import contextlib
    ctx = contextlib.ExitStack()
    pool_c = ctx.enter_context(tc.tile_pool(name="consts", bufs=1))
    pool_res = ctx.enter_context(tc.tile_pool(name="xres", bufs=1))
    pool_str = ctx.enter_context(tc.tile_pool(name="xstr", bufs=1))
    pool_xt = ctx.enter_context(tc.tile_pool(name="xt", bufs=2))
    pool_wA = ctx.enter_context(tc.tile_pool(name="wA", bufs=2))
    pool_wB = ctx.enter_context(tc.tile_pool(name="wB", bufs=2))
    pool_wC = ctx.enter_context(tc.tile_pool(name="wC", bufs=2))
    pool_s = ctx.enter_context(tc.tile_pool(name="state", bufs=1))
    pool_t = ctx.enter_context(tc.tile_pool(name="tmp", bufs=2))
    psum = ctx.enter_context(tc.tile_pool(name="ps", bufs=1,
                                          space=bass.MemorySpace.PSUM))

    def dma(dst, src):
        nc.sync.dma_start(out=dst, in_=src)

    # ---- constants -------------------------------------------------------
    ident = pool_c.tile([128, 128], F32, tag="ident")
    dma(ident[:], P["ident"][:])
    identb = pool_c.tile([128, 128], BF16, tag="identb")
    nc.vector.tensor_copy(identb[:], ident[:])
    poolw = pool_c.tile([128, DBLK], BF16, tag="poolw")
    dma(poolw[:], P["poolw"].rearrange("(c p) -> p c", p=128))
    onescol = pool_c.tile([128, 1], F32, tag="ones")
    nc.vector.memset(onescol[:], 1.0 / D)
    onesrow = pool_c.tile([1, 128], F32, tag="onesrow")
    nc.vector.memset(onesrow[:], 1.0)
    zeroc = pool_c.tile([128, 1], F32, tag="zeroc")
    nc.vector.memset(zeroc[:], 0.0)
    nc.const_aps.aps[(F32, 0.0)] = zeroc[:]
    epsc = pool_c.tile([128, 1], F32, tag="epsc")
    nc.vector.memset(epsc[:], EPS)
    nc.const_aps.aps[(F32, EPS)] = epsc[:]

    # classifier weights (loaded once)
    f1w = pool_c.tile([128, DBLK, 256], BF16, tag="f1w")
    dma(f1w[:], P["f1w"].rearrange("(c p) n -> p c n", p=128))
    f2w = pool_c.tile([128, 2, 128], BF16, tag="f2w")
    dma(f2w[:], P["f2w"].rearrange("(c p) n -> p c n", p=128))
    clsw = pool_c.tile([128, NCLS], BF16, tag="clsw")
    dma(clsw[:], P["clsw"][:])
    f1s = pool_c.tile([128, 2], F32, tag="f1s")
    dma(f1s[:], P["f1s"].rearrange("(c p) -> p c", p=128))
    f1b = pool_c.tile([128, 2], F32, tag="f1b")
    dma(f1b[:], P["f1b"].rearrange("(c p) -> p c", p=128))
    f2s = pool_c.tile([128, 1], F32, tag="f2s")
    dma(f2s[:], P["f2s"].rearrange("(c p) -> p c", p=128))
    f2b = pool_c.tile([128, 1], F32, tag="f2b")
    dma(f2b[:], P["f2b"].rearrange("(c p) -> p c", p=128))
    clsb = pool_c.tile([NCLS, 1], F32, tag="clsb")
    dma(clsb[:], P["clsb"].rearrange("(p c) -> p c", c=1))
    vecs = pool_c.tile([128, T, 8, DBLK], F32, tag="vecs")
    dma(vecs[:], P["vecs8"].rearrange("t v (c p) -> p t v c", p=128))
    bqv = pool_c.tile([DH, T, H], F32, tag="bqv")
    dma(bqv[:], P["bq"].rearrange("t p h -> p t h"))

    # persistent state
    zT = pool_s.tile([128, BL, DBLK], F32, tag="zT")
    zTb = pool_s.tile([128, BL, DBLK], BF16, tag="zTb")
    y1T = pool_s.tile([128, BL, DBLK], F32, tag="y1T")   # yT then z1T home
    y2T = pool_s.tile([128, BL, DBLK], F32, tag="y2T")   # y2T then z2T home
    z1Tb = pool_s.tile([128, BL, DBLK], BF16, tag="z1Tb")
    logitsT = pool_s.tile([NCLS, BL], F32, tag="logits")
    cT = pool_s.tile([128, DBLK, NGRP, GB * H], BF16, tag="cT")

    # resident xn rows (load now, keep forever)
    resident = {}
    for b in range(N_RES):
        xn = pool_res.tile([128, SCH, D], BF16, tag=f"xnr{b}", name=f"xnr{b}")
        dma(xn[:], P["xn"][b].rearrange("sc p d -> p sc d"))
        resident[b] = xn
    # rows with the transposed layout also resident (no per-pass transposes)
    xtres = {}
    for b in range(XT_RES):
        xtr = pool_res.tile([128, DBLK, S], BF16, tag=f"xtr{b}", name=f"xtr{b}")
        dma(xtr[:], P["xt"][b].rearrange("dc p s -> p dc s"))
        xtres[b] = xtr

    def vslice(t, k, dc):
        return vecs[:, t, VKEYS.index(k), dc:dc + 1]

    # ------------------------------------------------------------------
    # helpers
    # ------------------------------------------------------------------

    def ensure_xn(b):
        if b in resident:
            return resident[b]
        xn = pool_str.tile([128, SCH, D], BF16, tag=f"xs{b % N_SLOT}",
                           name=f"xs{b}")
        dma(xn[:], P["xn"][b].rearrange("sc p d -> p sc d"))
        return xn

    def make_xt_slice(xns, gi, dc):
        """PE-transpose dc-slice of 4 rows -> 4 xt sub-tiles [128, 512]."""
        outs = []
        for bb in range(GB):
            pt = psum.tile([128, SCH, 128], BF16, tag=f"tr{bb % 2}",
                           bufs=1)
            for sc in range(SCH):
                nc.tensor.transpose(pt[:, sc, :],
                                    xns[bb][:, sc, dc * 128:(dc + 1) * 128],
                                    identb[:])
            xts = pool_xt.tile([128, 512], BF16, tag=f"xts{bb}",
                               name=f"xt_g{gi}_d{dc}_b{bb}", bufs=3)
            nc.vector.tensor_copy(xts[:], pt[:].rearrange("p sc c -> p (sc c)"))
            outs.append(xts)
        return outs

    def stream_phase1(gi, score_lhsT, pool_mode):
        """Transposes + col-tiled scores + softmax for rows gi*4..gi*4+3.
        Emitted a group AHEAD of phase2 so the PE stays busy during the
        scalar/DVE softmax chain of the previous group."""
        bs = list(range(gi * GB, (gi + 1) * GB))
        xns = [ensure_xn(b) for b in bs]
        M = 1 if pool_mode else H
        ps = psum.tile([128, S], F32, tag="sc", bufs=2)
        for dc in range(DBLK):
            if bs[0] in xtres:
                xts = [xtres[b][:, dc, :] for b in bs]
            else:
                xts = [x[:] for x in make_xt_slice(xns, gi, dc)]
            for bb in range(GB):
                nc.tensor.matmul(ps[32 * bb:32 * bb + M, :],
                                 score_lhsT(dc, bb, bs[bb]), xts[bb],
                                 start=(dc == 0), stop=(dc == DBLK - 1),
                                 tile_position=(0, 32 * bb))
        gp = 32 * (GB - 1) + M
        es = pool_t.tile([GP, S], BF16, tag="att_e")
        den = pool_t.tile([GP, 2], F32, tag="att_d")
        nc.scalar.activation(es[:gp, :], ps[0:gp, :], AF.Exp,
                             accum_out=den[:gp, 0:1])
        nc.vector.reciprocal(den[:gp, 1:2], den[:gp, 0:1])
        att = pool_t.tile([GP, S], BF16, tag="att_n")
        nc.vector.tensor_scalar_mul(att[:gp, :], es[:gp, :], den[:gp, 1:2])
        return dict(gi=gi, att=att, xns=xns, M=M, gp=gp)

    def stream_phase2(st):
        """attT transpose + col-tiled weighted sums + cT-transpose evac.
        Returns ctp psum tile [128, DBLK, GP] bf16 (cols 32*bb+m live)."""
        att, xns, M, gp = st["att"], st["xns"], st["M"], st["gp"]
        ptr = psum.tile([128, SCH, GP], BF16, tag="ctr", bufs=1)
        for sc in range(SCH):
            nc.tensor.transpose(ptr[:, sc, :gp],
                                att[:gp, sc * 128:(sc + 1) * 128],
                                identb[:gp, :gp])
        attT = pool_t.tile([128, SCH, GP], BF16, tag="attT")
        nc.vector.tensor_copy(attT[:, :, :gp], ptr[:, :, :gp])
        c1 = psum.tile([128, 512], F32, tag="ws1", bufs=1)
        c2 = psum.tile([128, 256], F32, tag="ws2", bufs=1)
        for sc in range(SCH):
            for bb in range(GB):
                nc.tensor.matmul(c1[32 * bb:32 * bb + M, :],
                                 attT[:, sc, 32 * bb:32 * bb + M],
                                 xns[bb][:, sc, 0:512],
                                 start=(sc == 0), stop=(sc == SCH - 1),
                                 tile_position=(0, 32 * bb))
                nc.tensor.matmul(c2[32 * bb:32 * bb + M, :],
                                 attT[:, sc, 32 * bb:32 * bb + M],
                                 xns[bb][:, sc, 512:D],
                                 start=(sc == 0), stop=(sc == SCH - 1),
                                 tile_position=(0, 32 * bb))
        return evac_group_T(st["gi"], c1, c2, gp)

    def evac_group_T(gi, c1, c2, gp):
        """Evac c1/c2 [gp,512/256] -> sbuf bf16, transpose per dc -> psum;
        returns ctp psum tile [128, DBLK, GP] bf16 (cols 32*bb+m live)."""
        ch = pool_t.tile([GP, D], BF16, tag="chat")
        nc.vector.tensor_copy(ch[:gp, 0:512], c1[:gp, :])
        nc.vector.tensor_copy(ch[:gp, 512:D], c2[:gp, :])
        ctp = psum.tile([128, DBLK, GP], BF16, tag="ctr", bufs=1)
        for dc in range(DBLK):
            nc.tensor.transpose(ctp[:, dc, :gp],
                                ch[:gp, dc * 128:(dc + 1) * 128],
                                identb[:gp, :gp])
        return ctp

    # ==================================================================
    # pool pass (groups software-pipelined: phase1(g+1) before phase2(g))
    # ==================================================================
    def finish_pool(st):
        ctp = stream_phase2(st)
        for bb in range(GB):
            nc.vector.tensor_copy(zT[:, st["gi"] * GB + bb, :],
                                  ctp[:, :, 32 * bb])

    prev = None
    for gi in range(NGRP):
        cur = stream_phase1(gi, lambda dc, bb, b: poolw[:, dc:dc + 1],
                            pool_mode=True)
        if prev is not None:
            finish_pool(prev)
        prev = cur
    finish_pool(prev)

    nc.vector.tensor_copy(zTb[:], zT[:])

    # ==================================================================
    # transformer blocks
    # ==================================================================
    wscl = scal["wscl"]
    for t in range(T):
        g = scal["g"][t]
        sa = scal["sa"][t]
        s_q, s_k, s_v, s_o, s_c = wscl[t]

        # --- stream per-block weights (double-buffered via pools) ---
        wq = pool_wA.tile([128, DBLK, D], WDT, tag="wjq", name=f"wq{t}")
        dma(wq[:], P["wqT"][t].rearrange("(c p) n -> p c n", p=128))
        wk = pool_wB.tile([DH, H, D], WDT, tag="khk", name=f"wk{t}")
        dma(wk[:], P["wk"][t].rearrange("(h p) n -> p h n", p=DH))

        # --- q^T = wqT^T @ zT + bq  -> [96, 8, 32] ---
        qp = psum.tile([DH, H, BL], F32, tag="mm")
        for h in range(H):
            for jc in range(DBLK):
                nc.tensor.matmul(qp[:, h, :],
                                 wq[:, jc, h * DH:(h + 1) * DH],
                                 zTb[:, :, jc],
                                 start=(jc == 0), stop=(jc == DBLK - 1))
        qT = pool_t.tile([DH, H, BL], BF16, tag="qT", bufs=1)
        for h in range(H):
            nc.scalar.activation(qT[:, h, :], qp[:, h, :], AF.Identity,
                                 bias=bqv[:, t, h:h + 1], scale=s_q)

        # --- u = wk_h^T q_h -> [128, 6 dc, 8 h, 32 b], cast bf16 ---
        uT = pool_t.tile([128, DBLK, H, BL], BF16, tag="uT", bufs=1)
        for dc in range(DBLK):
            up = psum.tile([128, H, BL], F32, tag="mm")
            for h in range(H):
                nc.tensor.matmul(up[:, h, :],
                                 wk[:, h, dc * 128:(dc + 1) * 128],
                                 qT[:, h, :], start=True, stop=True)
            nc.scalar.activation(uT[:, dc, :, :], up[:], AF.Identity,
                                 scale=s_k)

        # prefetch tail weights while streaming
        wv = pool_wA.tile([128, DBLK, D], WDT, tag="wjq", name=f"wv{t}")
        dma(wv[:], P["wvT"][t].rearrange("(c p) n -> p c n", p=128))
        wo = pool_wB.tile([DH, H, D], WDT, tag="khk", name=f"wo{t}")
        dma(wo[:], P["woT"][t].rearrange("(h p) n -> p h n", p=DH))
        cdn = pool_wC.tile([128, DBLK, D], WDT, tag="cd", name=f"cdn{t}")
        dma(cdn[:], P["cden"][t].rearrange("(c p) n -> p c n", p=128))

        # --- streaming pass over x (software-pipelined groups) ---
        def finish_blk(st):
            ctp = stream_phase2(st)
            for bb in range(GB):
                nc.vector.tensor_copy(
                    cT[:, :, st["gi"], bb * H:(bb + 1) * H],
                    ctp[:, :, 32 * bb:32 * bb + H])

        prev = None
        for gi in range(NGRP):
            cur = stream_phase1(gi, lambda dc, bb, b: uT[:, dc, :, b],
                                pool_mode=False)
            if prev is not None:
                finish_blk(prev)
            prev = cur
        finish_blk(prev)

        # --- o_h = wvT_h^T @ c_h -> [96, 8, 32] ---
        op = psum.tile([DH, H, BL], F32, tag="mm")
        for h in range(H):
            for jc in range(DBLK):
                nc.tensor.matmul(
                    op[:, h, :],
                    wv[:, jc, h * DH:(h + 1) * DH],
                    cT[:, jc, :, :].rearrange("p g (bb h) -> p (g bb) h", bb=GB)[:, :, h],
                    start=(jc == 0), stop=(jc == DBLK - 1))
        oT = pool_t.tile([DH, H, BL], BF16, tag="oT", bufs=1)
        nc.scalar.activation(oT[:], op[:], AF.Identity, scale=s_v)

        # --- z_attn^T = woT^T @ o  (+ sa*bo_eff, * sa), y = z + ... ---
        zap = psum.tile([128, DBLK, BL], F32, tag="mm")
        for mk in range(DBLK):
            for h in range(H):
                nc.tensor.matmul(zap[:, mk, :],
                                 wo[:, h, mk * 128:(mk + 1) * 128],
                                 oT[:, h, :], start=(h == 0), stop=(h == H - 1))
        for mk in range(DBLK):
            nc.scalar.activation(y1T[:, :, mk], zap[:, mk, :], AF.Identity,
                                 bias=vslice(t, "bo_sa", mk), scale=sa * s_o)
        nc.vector.tensor_add(y1T[:], y1T[:], zT[:])

        # --- LN1 -> z1T (in place over y1T) ---
        _layernorm(nc, tc, psum, pool_t, y1T, y1T, onescol, onesrow,
                   lambda dc: vslice(t, "ln1_g", dc),
                   lambda dc: vslice(t, "ln1_b", dc))
        z1T = y1T
        nc.vector.tensor_copy(z1Tb[:], z1T[:])

        # --- denoise: z_den^T = cden^T @ z1T ; y2 = z1 + sd*(...) ---
        dp = psum.tile([128, DBLK, BL], F32, tag="mm")
        for mk in range(DBLK):
            for jc in range(DBLK):
                nc.tensor.matmul(dp[:, mk, :],
                                 cdn[:, jc, mk * 128:(mk + 1) * 128],
                                 z1Tb[:, :, jc],
                                 start=(jc == 0), stop=(jc == DBLK - 1))
        for mk in range(DBLK):
            nc.scalar.activation(y2T[:, :, mk], dp[:, mk, :], AF.Identity,
                                 bias=vslice(t, "projb_sd", mk), scale=s_c)
        nc.vector.tensor_add(y2T[:], y2T[:], z1T[:])

        # --- LN2 -> z2T (in place over y2T) ---
        _layernorm(nc, tc, psum, pool_t, y2T, y2T, onescol, onesrow,
                   lambda dc: vslice(t, "ln2_g", dc),
                   lambda dc: vslice(t, "ln2_b", dc))
        z2T = y2T

        # --- gate mix + BN -> new z ---
        nc.vector.tensor_sub(z2T[:], z2T[:], zT[:])       # z2 - z
        nc.vector.tensor_scalar(z2T[:], z2T[:], g, None,
                                op0=mybir.AluOpType.mult)  # g*(z2-z)
        nc.vector.tensor_add(z2T[:], z2T[:], zT[:])       # + z
        for dc in range(DBLK):
            nc.scalar.activation(zT[:, :, dc], z2T[:, :, dc], AF.Identity,
                                 bias=vslice(t, "bn_b", dc),
                                 scale=vslice(t, "bn_s", dc))

        # --- classifier ---
        nc.vector.tensor_copy(zTb[:], zT[:])
        hp = psum.tile([128, 2, BL], F32, tag="mm")
        for mk in range(2):
            for jc in range(DBLK):
                nc.tensor.matmul(hp[:, mk, :],
                                 f1w[:, jc, mk * 128:(mk + 1) * 128],
                                 zTb[:, :, jc],
                                 start=(jc == 0), stop=(jc == DBLK - 1))
        h1 = pool_t.tile([128, 2, BL], BF16, tag="h1")
        for mk in range(2):
            nc.scalar.activation(h1[:, mk, :], hp[:, mk, :], AF.Relu,
                                 bias=f1b[:, mk:mk + 1], scale=f1s[:, mk:mk + 1])
        h2p = psum.tile([128, BL], F32, tag="mm")
        for jc in range(2):
            nc.tensor.matmul(h2p[:], f2w[:, jc, :], h1[:, jc, :],
                             start=(jc == 0), stop=(jc == 1))
        h2 = pool_t.tile([128, BL], BF16, tag="h2")
        nc.scalar.activation(h2[:], h2p[:], AF.Relu,
                             bias=f2b[:, 0:1], scale=f2s[:, 0:1])
        lp = psum.tile([NCLS, BL], F32, tag="mm")
        nc.tensor.matmul(lp[:], clsw[:], h2[:], start=True, stop=True)
        if t == 0:
            nc.vector.tensor_copy(logitsT[:], lp[:])
        else:
            nc.vector.tensor_add(logitsT[:], logitsT[:], lp[:])

    # --- epilogue: /T + cls_b, store ---
    outt = pool_t.tile([NCLS, BL], F32, tag="outt")
    nc.scalar.activation(outt[:], logitsT[:], AF.Identity,
                         bias=clsb[:, 0:1], scale=1.0 / T)
    dma(P["out"][:], outt[:])
    ctx.close()


def _layernorm(nc, tc, psum, pool_t, yT, outT, onescol, onesrow, gf, bf):
    """T-layout layernorm over d (partition x dblk)."""
    lnp = psum.tile([1, 2, BL], F32, tag="mm")
    mp = lnp[:, 0, :]
    m2p = lnp[:, 1, :]
    sq = pool_t.tile([128, BL, DBLK], F32, tag="ln_sq", bufs=1)
    nc.scalar.square(sq[:], yT[:])
    # NOTE: mp and m2p share a PSUM bank row; interleaving their
    # accumulation groups is illegal (start=True clears has_written for
    # the whole partition-row of the bank) -> run mp's group to
    # completion before starting m2p's.
    for dc in range(DBLK):
        nc.tensor.matmul(mp[:], onescol[:], yT[:, :, dc],
                         start=(dc == 0), stop=(dc == DBLK - 1))
    for dc in range(DBLK):
        nc.tensor.matmul(m2p[:], onescol[:], sq[:, :, dc],
                         start=(dc == 0), stop=(dc == DBLK - 1))
    st = pool_t.tile([1, 2 * BL], F32, tag="ln_st")  # [mu | rstd]
    nc.vector.tensor_copy(st[:, 0:BL], mp[:])
    mu2 = pool_t.tile([1, BL], F32, tag="ln_mu2")
    nc.scalar.square(mu2[:], st[:, 0:BL])
    var = pool_t.tile([1, BL], F32, tag="ln_var")
    nc.vector.tensor_sub(var[:], m2p[:], mu2[:])
    nc.scalar.activation(var[:], var[:], AF.Sqrt, bias=EPS)
    nc.vector.reciprocal(st[:, BL:2 * BL], var[:])
    bcp = psum.tile([128, 2 * BL], F32, tag="ctr", bufs=1)
    nc.tensor.matmul(bcp[:], onesrow[:], st[:], start=True, stop=True)
    bc = pool_t.tile([128, 2 * BL], F32, tag="ln_bc")
    nc.vector.tensor_copy(bc[:], bcp[:])
    mub = bc[:, 0:BL]
    rsb = bc[:, BL:2 * BL]
    for dc in range(DBLK):
        nc.vector.tensor_sub(outT[:, :, dc], yT[:, :, dc], mub)
        nc.vector.tensor_mul(outT[:, :, dc], outT[:, :, dc], rsb)
        nc.scalar.activation(outT[:, :, dc], outT[:, :, dc], AF.Identity,
                             bias=bf(dc), scale=gf(dc))


# ---------------------------------------------------------------------------
# entry point
# ---------------------------------------------------------------------------

_PROG_CACHE = {}
LAST_EXEC_NS = None
LAST_RESULT = None


def _make_in_maps(inputs, w, scal):
    x = np.asarray(inputs["x_feat"], np.float32)
    assert x.shape == (B, S, D), x.shape
    bf = _np_dt(BF16)

    shared = {
        "ident": w["ident"],
        "poolw": w["poolw"].astype(bf),
        "wqT": w["wqT"], "bq": w["bq"], "wk": w["wk"],
        "wvT": w["wvT"], "woT": w["woT"], "cden": w["cden"],
        "vecs8": w["vecs8"],
        "f1w": w["f1w"].astype(bf), "f1s": w["f1s"], "f1b": w["f1b"],
        "f2w": w["f2w"].astype(bf), "f2s": w["f2s"], "f2b": w["f2b"],
        "clsw": w["clsw"].astype(bf), "clsb": w["clsb"],
    }
    in_maps = []
    for c in range(NCORES):
        m = dict(shared)
        xc = x[c * BL:(c + 1) * BL].astype(bf)            # [BL, S, D]
        m["xn"] = np.ascontiguousarray(
            xc.reshape(BL, SCH, 128, D))
        m["xt"] = np.ascontiguousarray(
            xc[:XT_RES].transpose(0, 2, 1).reshape(XT_RES, DBLK, 128, S))
        in_maps.append(m)
    return in_maps


def kernel(**inputs):
    global LAST_EXEC_NS, LAST_RESULT
    w, scal = _prep_weights(inputs)
    key = (tuple(scal["g"]) + tuple(scal["sa"])
           + tuple(x for row in scal["wscl"] for x in row))
    if key not in _PROG_CACHE:
        _PROG_CACHE[key] = build_program(scal)
    nc = _PROG_CACHE[key]
    in_maps = _make_in_maps(inputs, w, scal)
    res = run_bass_kernel_spmd(nc, in_maps, core_ids=list(range(NCORES)))
    LAST_RESULT = res
    if getattr(res, "exec_time_ns", None):
        LAST_EXEC_NS = res.exec_time_ns
    out = np.concatenate(
        [np.asarray(res.results[c]["out"]).T for c in range(NCORES)], axis=0)
    return out.astype(np.float32)
